# revision 3
# baseline (speedup 1.0000x reference)
"""Dense dot-product attention (B=32, S=2048, D=128, fp32) on 8 TRN2 cores.

Sharding: batch dim B=32 split across 8 cores (4 batches/core); each core
computes full S x S attention for its batches independently (no collectives).

v2: host pre-converts Q,K,V to fp16 (plenty of accuracy headroom vs the
2e-2 gate) so the device does zero input casting and DMA traffic halves.
Per-core kernel, per batch ("S^T layout", k on partitions):
  for each q-phase (1024 wide), for each k-chunk j (16 x 128):
    S^T_j = Kt_j.T @ Qt[:, phase]      (PE, fp16 -> PSUM fp32)
    P^T_j = exp(scale * S^T_j)         (ACT, PSUM -> SBUF fp16)
    acc  += P^T_j                      (DVE fp16, 2x rate)
    O^T  += V_j.T @ P^T_j              (PE, PSUM fp32 [128d, q])
  drain: l = partition_all_reduce(acc) (GPSIMD, result on all partitions);
  1/l (DVE fast reciprocal, all partitions); O^T * (1/l) fused from PSUM ->
  SBUF fp16 (DVE); DMA out fp16 (host upcasts to fp32).
ACT runs only the exp — it is the bottleneck engine (~1.04us per [128,1024]
tile, 128 tiles); everything else is sized to stay under it.
"""

import sys

if "/opt/trn_rl_repo" not in sys.path:
    sys.path.insert(0, "/opt/trn_rl_repo")

import numpy as np

import concourse.bacc as bacc
import concourse.mybir as mybir
import concourse.tile as tile
from concourse import bass_utils
from concourse import bass_isa

N_CORES = 8
B = 32
S = 2048
D = 128
P = 128
BPC = B // N_CORES          # batches per core = 4
NJ = S // P                 # 16 k-chunks of 128
QH = 1024                   # q-phase width
NPH = S // QH               # 2 phases
NC_ = 512                   # matmul moving-operand chunk
SCALE = 1.0 / float(np.sqrt(D))

f32 = mybir.dt.float32
f16 = mybir.dt.float16
EXP = mybir.ActivationFunctionType.Exp


def build(repeat=1):
    """repeat>1 duplicates the whole per-core workload (same inputs/outputs)
    back-to-back inside one NEFF — used only for differential wall-clock
    timing of the hardware kernel (host/dispatch overhead cancels)."""
    nc = bacc.Bacc("TRN2", target_bir_lowering=False, debug=False)

    Qtd = nc.dram_tensor("Qt", [BPC, D, S], f16, kind="ExternalInput")
    Ktd = nc.dram_tensor("Kt", [BPC, D, S], f16, kind="ExternalInput")
    Vd = nc.dram_tensor("V_p", [BPC, S, D], f16, kind="ExternalInput")
    Otd = nc.dram_tensor("Ot", [BPC, D, S], f16, kind="ExternalOutput")

    with tile.TileContext(nc) as tc:
        with (
            tc.tile_pool(name="inp", bufs=3) as in_pool,
            tc.tile_pool(name="pt", bufs=8) as pt_pool,
            tc.tile_pool(name="misc", bufs=2) as misc_pool,
            tc.tile_pool(name="ot", bufs=2) as ot_pool,
            tc.tile_pool(name="acc", bufs=2) as acc_pool,
            tc.tile_pool(name="s_ps", bufs=2, space="PSUM") as s_pool,
            tc.tile_pool(name="o_ps", bufs=2, space="PSUM") as o_pool,
        ):
            inputs = {}
            NB = BPC * repeat

            def load_batch(bi):
                b = bi % BPC
                qt = in_pool.tile([P, S], f16, tag="qt")
                kt = in_pool.tile([P, S], f16, tag="kt")
                v_r = in_pool.tile([P, NJ, D], f16, tag="v_r")
                v_src = Vd[b].rearrange("(n p) d -> p n d", p=P)
                # head chunks first so compute can start early
                nc.sync.dma_start(kt[:, :256], Ktd[b, :, :256])
                nc.sync.dma_start(qt[:, :QH], Qtd[b, :, :QH])
                nc.sync.dma_start(v_r[:, :NJ // 2], v_src[:, :NJ // 2])
                nc.sync.dma_start(kt[:, 256:], Ktd[b, :, 256:])
                nc.sync.dma_start(qt[:, QH:], Qtd[b, :, QH:])
                nc.sync.dma_start(v_r[:, NJ // 2:], v_src[:, NJ // 2:])
                inputs[bi] = (qt, kt, v_r)

            load_batch(0)

            iters = [
                (bi, h, j)
                for bi in range(NB)
                for h in range(NPH)
                for j in range(NJ)
            ]
            T = len(iters)

            def emit_scores(t):
                bi, h, j = iters[t]
                qt, kt, _ = inputs[bi]
                s_ps = s_pool.tile([P, QH], f32, tag="s")
                for c in range(QH // NC_):
                    nc.tensor.matmul(
                        s_ps[:, c * NC_:(c + 1) * NC_],
                        kt[:, j * P:(j + 1) * P],
                        qt[:, h * QH + c * NC_: h * QH + (c + 1) * NC_],
                        start=True, stop=True,
                    )
                return s_ps

            s_next = emit_scores(0)
            o_ps = acc = None
            for t in range(T):
                bi, h, j = iters[t]
                b = bi % BPC
                if j == 0:
                    o_ps = o_pool.tile([P, QH], f32, tag="o")
                    acc = acc_pool.tile([P, QH], f16, tag="acc")
                s_ps = s_next
                pt = pt_pool.tile([P, QH], f16, tag="pt")
                nc.scalar.activation(pt[:], s_ps[:], EXP, scale=SCALE)
                # prefetch the next batch's inputs a full batch ahead
                if h == 0 and j == 2 and bi + 1 < NB:
                    load_batch(bi + 1)
                # software pipeline: issue the next scores matmuls ahead of
                # this iteration's PSUM-consumers so the in-order PE never
                # stalls on the ACT result.
                if t + 1 < T:
                    s_next = emit_scores(t + 1)
                # row sums: accumulate exp tiles on the DVE (j-partials) in
                # fp16 (2x rate); cross-partition reduction once per phase
                # on GPSIMD.
                if j == 0:
                    nc.vector.tensor_copy(acc[:], pt[:])
                else:
                    nc.vector.tensor_add(acc[:], acc[:], pt[:])
                for c in range(QH // NC_):
                    nc.tensor.matmul(
                        o_ps[:, c * NC_:(c + 1) * NC_],
                        inputs[bi][2][:, j, :],
                        pt[:, c * NC_:(c + 1) * NC_],
                        start=(j == 0), stop=(j == NJ - 1),
                    )
                if j == NJ - 1:
                    # softmax denominators: sum acc across partitions on the
                    # (otherwise idle) GPSIMD; result lands on all partitions
                    lsum = misc_pool.tile([P, QH], f32, tag="lsum")
                    nc.gpsimd.partition_all_reduce(
                        lsum[:], acc[:], channels=P,
                        reduce_op=bass_isa.ReduceOp.add,
                    )
                    recip = misc_pool.tile([P, QH], f32, tag="recip")
                    nc.vector.reciprocal_approx_fast(recip[:], lsum[:])
                    # fused drain+normalize: O^T read straight from PSUM
                    # (o_ps is double-buffered so the next phase's PV isn't
                    # blocked on this)
                    ot = ot_pool.tile([P, QH], f16, tag="ot")
                    nc.vector.tensor_mul(ot[:], o_ps[:], recip[:])
                    nc.sync.dma_start(Otd[b, :, h * QH:(h + 1) * QH], ot[:])

    nc.compile()
    return nc


_nc_cache = None


def _get_nc():
    global _nc_cache
    if _nc_cache is None:
        _nc_cache = build()
    return _nc_cache


def make_in_maps(Q_p, K_p, V_p):
    """Host-side shard prep: transpose Q,K to [B, D, S], cast all to fp16,
    split across cores."""
    Qt = np.ascontiguousarray(
        np.asarray(Q_p, dtype=np.float32).transpose(0, 2, 1)
    ).astype(np.float16)
    Kt = np.ascontiguousarray(
        np.asarray(K_p, dtype=np.float32).transpose(0, 2, 1)
    ).astype(np.float16)
    V = np.asarray(V_p, dtype=np.float32).astype(np.float16)
    return [
        {
            "Qt": Qt[c * BPC:(c + 1) * BPC],
            "Kt": Kt[c * BPC:(c + 1) * BPC],
            "V_p": V[c * BPC:(c + 1) * BPC],
        }
        for c in range(N_CORES)
    ]


def kernel(Q_p, K_p, V_p, trace=False):
    in_maps = make_in_maps(Q_p, K_p, V_p)
    nc = _get_nc()
    try:
        res = bass_utils.run_bass_kernel_spmd(
            nc, in_maps, core_ids=list(range(N_CORES)), trace=trace
        )
    except Exception:
        # shared terminals occasionally throw transient NRT errors; retry once
        import time as _time
        _time.sleep(5)
        res = bass_utils.run_bass_kernel_spmd(
            nc, in_maps, core_ids=list(range(N_CORES)), trace=trace
        )
    out = np.empty((B, S, D), dtype=np.float32)
    for c in range(N_CORES):
        ot = res.results[c]["Ot"]  # [BPC, D, S] fp16
        out[c * BPC:(c + 1) * BPC] = ot.transpose(0, 2, 1).astype(np.float32)
    if trace:
        kernel.last_exec_time_ns = res.exec_time_ns
        kernel.last_results = res
    return out


# revision 4
# speedup vs baseline: 1.1273x; 1.1273x over previous
"""Dense dot-product attention (B=32, S=2048, D=128, fp32) on 8 TRN2 cores.

Sharding: batch dim B=32 split across 8 cores (4 batches/core); each core
computes full S x S attention for its batches independently (no collectives).

v2: host pre-converts Q,K,V to fp16 (plenty of accuracy headroom vs the
2e-2 gate) so the device does zero input casting and DMA traffic halves.
Per-core kernel, per batch ("S^T layout", k on partitions):
  for each q-phase (1024 wide), for each k-chunk j (16 x 128):
    S^T_j = Kt_j.T @ Qt[:, phase]      (PE, fp16 -> PSUM fp32)
    P^T_j = exp(scale * S^T_j)         (ACT, PSUM -> SBUF fp16)
    acc  += P^T_j                      (DVE fp16, 2x rate)
    O^T  += V_j.T @ P^T_j              (PE, PSUM fp32 [128d, q])
  drain: l = partition_all_reduce(acc) (GPSIMD, result on all partitions);
  1/l (DVE fast reciprocal, all partitions); O^T * (1/l) fused from PSUM ->
  SBUF fp16 (DVE); DMA out fp16 (host upcasts to fp32).
ACT runs only the exp — it is the bottleneck engine (~1.04us per [128,1024]
tile, 128 tiles); everything else is sized to stay under it.
"""

import sys

if "/opt/trn_rl_repo" not in sys.path:
    sys.path.insert(0, "/opt/trn_rl_repo")

import numpy as np

import concourse.bacc as bacc
import concourse.mybir as mybir
import concourse.tile as tile
from concourse import bass_utils
from concourse import bass_isa

N_CORES = 8
B = 32
S = 2048
D = 128
P = 128
BPC = B // N_CORES          # batches per core = 4
NJ = S // P                 # 16 k-chunks of 128
QH = 1024                   # q-phase width
NPH = S // QH               # 2 phases
NC_ = 512                   # matmul moving-operand chunk
SCALE = 1.0 / float(np.sqrt(D))

f32 = mybir.dt.float32
f16 = mybir.dt.bfloat16  # bf16: PE double-pumps bf16 (fp16 runs half-rate)
EXP = mybir.ActivationFunctionType.Exp


def build(repeat=1):
    """repeat>1 duplicates the whole per-core workload (same inputs/outputs)
    back-to-back inside one NEFF — used only for differential wall-clock
    timing of the hardware kernel (host/dispatch overhead cancels)."""
    nc = bacc.Bacc("TRN2", target_bir_lowering=False, debug=False)

    Qtd = nc.dram_tensor("Qt", [BPC, D, S], f16, kind="ExternalInput")
    Ktd = nc.dram_tensor("Kt", [BPC, D, S], f16, kind="ExternalInput")
    Vd = nc.dram_tensor("V_p", [BPC, S, D], f16, kind="ExternalInput")
    Otd = nc.dram_tensor("Ot", [BPC, D, S], f16, kind="ExternalOutput")

    with tile.TileContext(nc) as tc:
        with (
            tc.tile_pool(name="inp", bufs=3) as in_pool,
            tc.tile_pool(name="pt", bufs=8) as pt_pool,
            tc.tile_pool(name="misc", bufs=2) as misc_pool,
            tc.tile_pool(name="ot", bufs=2) as ot_pool,
            tc.tile_pool(name="acc", bufs=2) as acc_pool,
            tc.tile_pool(name="s_ps", bufs=2, space="PSUM") as s_pool,
            tc.tile_pool(name="o_ps", bufs=2, space="PSUM") as o_pool,
        ):
            inputs = {}
            NB = BPC * repeat

            def load_batch(bi):
                b = bi % BPC
                qt = in_pool.tile([P, S], f16, tag="qt")
                kt = in_pool.tile([P, S], f16, tag="kt")
                v_r = in_pool.tile([P, NJ, D], f16, tag="v_r")
                v_src = Vd[b].rearrange("(n p) d -> p n d", p=P)
                # head chunks first so compute can start early
                nc.sync.dma_start(kt[:, :256], Ktd[b, :, :256])
                nc.sync.dma_start(qt[:, :QH], Qtd[b, :, :QH])
                nc.sync.dma_start(v_r[:, :NJ // 2], v_src[:, :NJ // 2])
                nc.sync.dma_start(kt[:, 256:], Ktd[b, :, 256:])
                nc.sync.dma_start(qt[:, QH:], Qtd[b, :, QH:])
                nc.sync.dma_start(v_r[:, NJ // 2:], v_src[:, NJ // 2:])
                inputs[bi] = (qt, kt, v_r)

            load_batch(0)

            iters = [
                (bi, h, j)
                for bi in range(NB)
                for h in range(NPH)
                for j in range(NJ)
            ]
            T = len(iters)

            def emit_scores(t):
                bi, h, j = iters[t]
                qt, kt, _ = inputs[bi]
                s_ps = s_pool.tile([P, QH], f32, tag="s")
                for c in range(QH // NC_):
                    nc.tensor.matmul(
                        s_ps[:, c * NC_:(c + 1) * NC_],
                        kt[:, j * P:(j + 1) * P],
                        qt[:, h * QH + c * NC_: h * QH + (c + 1) * NC_],
                        start=True, stop=True,
                    )
                return s_ps

            s_next = emit_scores(0)
            o_ps = acc = None
            for t in range(T):
                bi, h, j = iters[t]
                b = bi % BPC
                if j == 0:
                    o_ps = o_pool.tile([P, QH], f32, tag="o")
                    acc = acc_pool.tile([P, QH], f16, tag="acc")
                s_ps = s_next
                pt = pt_pool.tile([P, QH], f16, tag="pt")
                nc.scalar.activation(pt[:], s_ps[:], EXP, scale=SCALE)
                # prefetch the next batch's inputs a full batch ahead
                if h == 0 and j == 2 and bi + 1 < NB:
                    load_batch(bi + 1)
                # software pipeline: issue the next scores matmuls ahead of
                # this iteration's PSUM-consumers so the in-order PE never
                # stalls on the ACT result.
                if t + 1 < T:
                    s_next = emit_scores(t + 1)
                # row sums: accumulate exp tiles on the DVE (j-partials) in
                # fp16 (2x rate); cross-partition reduction once per phase
                # on GPSIMD.
                if j == 0:
                    nc.vector.tensor_copy(acc[:], pt[:])
                else:
                    nc.vector.tensor_add(acc[:], acc[:], pt[:])
                for c in range(QH // NC_):
                    nc.tensor.matmul(
                        o_ps[:, c * NC_:(c + 1) * NC_],
                        inputs[bi][2][:, j, :],
                        pt[:, c * NC_:(c + 1) * NC_],
                        start=(j == 0), stop=(j == NJ - 1),
                    )
                if j == NJ - 1:
                    # softmax denominators: sum acc across partitions on the
                    # (otherwise idle) GPSIMD; result lands on all partitions
                    lsum = misc_pool.tile([P, QH], f32, tag="lsum")
                    nc.gpsimd.partition_all_reduce(
                        lsum[:], acc[:], channels=P,
                        reduce_op=bass_isa.ReduceOp.add,
                    )
                    recip = misc_pool.tile([P, QH], f32, tag="recip")
                    nc.vector.reciprocal_approx_fast(recip[:], lsum[:])
                    # fused drain+normalize: O^T read straight from PSUM
                    # (o_ps is double-buffered so the next phase's PV isn't
                    # blocked on this)
                    ot = ot_pool.tile([P, QH], f16, tag="ot")
                    nc.vector.tensor_mul(ot[:], o_ps[:], recip[:])
                    nc.sync.dma_start(Otd[b, :, h * QH:(h + 1) * QH], ot[:])

    nc.compile()
    return nc


_nc_cache = None


def _get_nc():
    global _nc_cache
    if _nc_cache is None:
        _nc_cache = build()
    return _nc_cache


def make_in_maps(Q_p, K_p, V_p):
    """Host-side shard prep: transpose Q,K to [B, D, S], cast all to bf16,
    split across cores."""
    import ml_dtypes
    bf16 = ml_dtypes.bfloat16
    Qt = np.ascontiguousarray(
        np.asarray(Q_p, dtype=np.float32).transpose(0, 2, 1)
    ).astype(bf16)
    Kt = np.ascontiguousarray(
        np.asarray(K_p, dtype=np.float32).transpose(0, 2, 1)
    ).astype(bf16)
    V = np.asarray(V_p, dtype=np.float32).astype(bf16)
    return [
        {
            "Qt": Qt[c * BPC:(c + 1) * BPC],
            "Kt": Kt[c * BPC:(c + 1) * BPC],
            "V_p": V[c * BPC:(c + 1) * BPC],
        }
        for c in range(N_CORES)
    ]


def kernel(Q_p, K_p, V_p, trace=False):
    in_maps = make_in_maps(Q_p, K_p, V_p)
    nc = _get_nc()
    try:
        res = bass_utils.run_bass_kernel_spmd(
            nc, in_maps, core_ids=list(range(N_CORES)), trace=trace
        )
    except Exception:
        # shared terminals occasionally throw transient NRT errors; retry once
        import time as _time
        _time.sleep(5)
        res = bass_utils.run_bass_kernel_spmd(
            nc, in_maps, core_ids=list(range(N_CORES)), trace=trace
        )
    out = np.empty((B, S, D), dtype=np.float32)
    for c in range(N_CORES):
        ot = res.results[c]["Ot"]  # [BPC, D, S] fp16
        out[c * BPC:(c + 1) * BPC] = ot.transpose(0, 2, 1).astype(np.float32)
    if trace:
        kernel.last_exec_time_ns = res.exec_time_ns
        kernel.last_results = res
    return out
